# revision 36
# baseline (speedup 1.0000x reference)
"""Trainium2 Bass kernel for nn_PairwiseAttentionTerminal.

Reference computation (L=1024, B=8, F=256, H=8, C=32):
    x = layernorm(features)                       # (L, B, F)
    q,k,v = x@Wq+bq, x@Wk+bk, x@Wv+bv             # (L, B, H, C)
    bias  = x@Wb+bb                               # (L, B, H) per-key bias
    gate  = sigmoid(x@Wg+bg)                      # (L, B, H, C)
    S     = einsum('qbhc,kbhc->qbkh', q, k)/sqrt(C) + bias[None]
    attn  = softmax_k(S) @ v                      # (L, B, H, C)
    out   = (attn*gate) @ Wo + bo                 # (L, B, F)

Sharding: batch B=8 -> one batch element per NeuronCore (8 cores), weights
replicated, no collectives.  Host shards/gathers around one SPMD NEFF.

Per-core engine plan (cost-model driven):
  - ACT is the bottleneck: 64 softmax exps of [128k x 1024q] from PSUM
    (per-key bias = per-partition ACT bias operand).  Everything else is
    arranged to hide under that stream.
  - S^T[k,q] per (head, ktile): 32-contraction f32r matmuls (2 x 512 free).
  - AV restructured as out[q, c]: stationary = eT [128k x 128q] slice (bf16),
    moving = ones-augmented V [128k x 33] (bf16) -> 33-cycle matmuls into a
    single-bank accumulator [128, 8qq, 33]; denominator rides along as col 32.
    AV emission is software-pipelined one (h,kk) step behind the S/exp stream
    so the in-order PE queue never blocks on the current exp.
  - Normalize in q-major layout: DVE reciprocal of D*(1+e^-y) fuses the
    sigmoid gate division; no DRAM broadcast roundtrip.  Heads 0-3 normalize
    under the h4-6 exp stream, 4-6 under h7, only h7 in the tail.
  - gate/v/output biases are rank-1 matmuls (ones[1,128] x bias_row) chained
    into the projection accumulation.
  - ag (gated attn, q-major bf16) -> PE-transposed (bf16 identity, 1 cyc/row)
    -> Wo projection per qtile, pipelined drain+DMA tail.
  - PE heater matmuls at t=0 ramp the PE clock (p-state) before the real
    front (LN -> transpose -> q/k projections) hits it.
"""

import numpy as np
from contextlib import ExitStack

L, B, F, H, C = 1024, 8, 256, 8, 32
HC = H * C
C1 = C + 1
EPS = 1e-5
N_CORES = 8
P = 128
NLT = L // P   # 8 L-tiles (== qtiles == ktiles)
NFC = F // P   # 2 F-chunks
NQT = L // P   # 8 q-tiles

_COMPILED = {}


def _build():
    import concourse.bacc as bacc
    import concourse.mybir as mybir
    import concourse.tile as tile

    f32 = mybir.dt.float32
    f32r = mybir.dt.float32r
    bf16 = mybir.dt.bfloat16
    AF = mybir.ActivationFunctionType
    ALU = mybir.AluOpType

    nc = bacc.Bacc("TRN2", target_bir_lowering=False)

    # ---- DRAM I/O (per-core) ----
    feat_e = nc.dram_tensor("feat", [L, F], f32, kind="ExternalInput")
    wq_e = nc.dram_tensor("wq", [P, NFC, HC], f32r, kind="ExternalInput")
    wk_e = nc.dram_tensor("wk", [P, NFC, HC], f32r, kind="ExternalInput")
    wv_e = nc.dram_tensor("wv", [P, NFC, HC], f32r, kind="ExternalInput")
    wg_e = nc.dram_tensor("wg", [P, NFC, HC], f32r, kind="ExternalInput")
    wb_e = nc.dram_tensor("wb", [P, NFC, H], f32r, kind="ExternalInput")
    wo_e = nc.dram_tensor("wo", [P, NFC, F], f32r, kind="ExternalInput")
    bq_e = nc.dram_tensor("bq_t", [P, NFC], f32, kind="ExternalInput")
    bk_e = nc.dram_tensor("bk_t", [P, NFC], f32, kind="ExternalInput")
    bv_e = nc.dram_tensor("bv_r", [1, HC], f32r, kind="ExternalInput")
    bg_e = nc.dram_tensor("bg_r", [1, HC], f32r, kind="ExternalInput")
    bo_e = nc.dram_tensor("bo_r", [1, F], f32r, kind="ExternalInput")
    bb_e = nc.dram_tensor("bb_b", [P, H], f32, kind="ExternalInput")
    id_e = nc.dram_tensor("ident", [P, P], f32r, kind="ExternalInput")
    ones_e = nc.dram_tensor("ones1", [1, P], f32r, kind="ExternalInput")
    out_e = nc.dram_tensor("out", [L, F], f32, kind="ExternalOutput")

    with tile.TileContext(nc) as tc, ExitStack() as ctx:
        const = ctx.enter_context(tc.tile_pool(name="const", bufs=1))
        main = ctx.enter_context(tc.tile_pool(name="main", bufs=1))
        work = ctx.enter_context(tc.tile_pool(name="work", bufs=4))
        epool = ctx.enter_context(tc.tile_pool(name="epool", bufs=4))
        npool = ctx.enter_context(tc.tile_pool(name="npool", bufs=8))
        opool = ctx.enter_context(tc.tile_pool(name="opool", bufs=4))

        # ---- t=0: heater fuel + ACT table prewarm source ----
        ones512 = const.tile([P, 512], f32, name="ones512")
        nc.vector.memset(ones512[:], 1.0)

        # ---- input DMAs, ordered by first use ----
        ft = [const.tile([P, F], f32, name=f"ft{i}") for i in range(NLT)]
        for i in range(4):
            nc.sync.dma_start(ft[i][:], feat_e.ap()[i * P:(i + 1) * P, :])

        def load(eng, name, ext, shape, dt_=f32):
            t = const.tile(shape, dt_, name=name)
            eng.dma_start(t[:], ext.ap())
            return t

        wq = load(nc.sync, "wq_s", wq_e, [P, NFC, HC], f32r)
        wk = load(nc.sync, "wk_s", wk_e, [P, NFC, HC], f32r)
        bq = load(nc.sync, "bq_s", bq_e, [P, NFC])
        bk = load(nc.sync, "bk_s", bk_e, [P, NFC])
        bvr = load(nc.sync, "bv_s", bv_e, [1, HC], f32r)
        for i in range(4, NLT):
            nc.gpsimd.dma_start(ft[i][:], feat_e.ap()[i * P:(i + 1) * P, :])
        ident = load(nc.gpsimd, "id_s", id_e, [P, P], f32r)
        wb = load(nc.gpsimd, "wb_s", wb_e, [P, NFC, H], f32r)
        ones1 = load(nc.gpsimd, "ones1_s", ones_e, [1, P], f32r)
        wv = load(nc.gpsimd, "wv_s", wv_e, [P, NFC, HC], f32r)
        wg = load(nc.gpsimd, "wg_s", wg_e, [P, NFC, HC], f32r)
        bgr = load(nc.gpsimd, "bg_s", bg_e, [1, HC], f32r)
        wo = load(nc.sync, "wo_s", wo_e, [P, NFC, F], f32r)
        bor = load(nc.sync, "bo_s", bo_e, [1, F], f32r)

        # ACT table prewarm: one Ln on the memset-ones tile loads the
        # combined ln/exp table before the front needs it.
        scr0 = const.tile([P, 2], f32, name="scr0")
        nc.scalar.activation(scr0[:, 0:1], ones512[:, 0:1], AF.Ln)

        epst = const.tile([P, 1], f32, name="epst")
        nc.vector.memset(epst[:], EPS)

        # bf16 identity for the ag transposes (1 cyc/row vs 1.5 for f32r);
        # copied on Pool so it stays off the DVE LN stream
        identb = const.tile([P, P], bf16, name="identb")
        nc.gpsimd.tensor_copy(identb[:], ident[:])

        # ---- persistent tiles ----
        xT = [main.tile([P, L], f32r, name=f"xT{j}") for j in range(NFC)]
        qT = [main.tile([P, L], f32r, name=f"qT{j}") for j in range(NFC)]
        kT = [main.tile([P, L], f32r, name=f"kT{j}") for j in range(NFC)]
        vaug = [main.tile([P, H, C1], bf16, name=f"vaug{i}") for i in range(NLT)]
        u = [main.tile([P, H], f32, name=f"u{i}") for i in range(NLT)]
        ge = [main.tile([P, HC], bf16, name=f"ge{q}") for q in range(NQT)]
        g1A = [main.tile([P, 4, C], bf16, name=f"g1A{q}") for q in range(NQT)]
        g1B = [main.tile([P, 3, C], bf16, name=f"g1B{q}") for q in range(NQT)]
        g17 = [main.tile([P, 1, C], bf16, name=f"g17{q}") for q in range(NQT)]
        att = main.tile([P, NQT, H, C1], f32, name="att")
        ag = [main.tile([P, HC], bf16, name=f"ag{q}") for q in range(NQT)]
        agT = [main.tile([P, L], f32r, name=f"agT{j}") for j in range(NFC)]

        # ================= Front phase =================
        # psF: ftp 1 bank + fqk 1 + fsm 2 = 4 banks; psW (wave-1) 4 banks.
        psF_cm = tc.tile_pool(name="psF", bufs=1, space="PSUM")
        psF = psF_cm.__enter__()

        def heat(n):
            # heaters ride the fqk slot (same shape as qk psum tiles)
            for _ in range(n):
                hp_ = psF.tile([P, 512], f32, tag="fqk", name="heat", bufs=2)
                nc.tensor.matmul(hp_[:], ones512[:, 0:P].bitcast(f32r),
                                 ones512[:].bitcast(f32r),
                                 start=True, stop=True)

        heat(6)

        # LN per L-tile: stats (DVE-paced) split from finish so the DVE
        # stream never blocks on the cross-engine rstd wait
        def ln_stats(i):
            st = work.tile([P, 8], f32, tag="st", bufs=8)
            nc.vector.bn_stats(st[:, 0:6], ft[i][:])
            nc.vector.bn_aggr(st[:, 6:8], st[:, 0:6])
            # rstd = exp(-0.5*ln(var+eps)) (free: scalar ops cost 0)
            nc.scalar.activation(st[:, 3:4], st[:, 7:8], AF.Ln, bias=epst[:])
            nc.scalar.activation(st[:, 4:5], st[:, 3:4], AF.Exp, scale=-0.5)
            return st

        def ln_fin(i, st):
            xn = work.tile([P, F], f32r, tag="xn")
            nc.vector.tensor_scalar(xn[:], ft[i][:], st[:, 6:7], st[:, 4:5],
                                    op0=ALU.subtract, op1=ALU.mult)
            for j in range(NFC):
                tpw = psF.tile([P, 512], f32r, tag="fqk", name=f"tp{i}_{j}",
                               bufs=2)
                tp = tpw[:, 0:P]
                nc.tensor.transpose(tp, xn[:, j * P:(j + 1) * P], ident[:])
                # alternate drains ACT/DVE (ACT idle during the front)
                if (2 * i + j) % 2 == 0:
                    nc.scalar.activation(xT[j][:, i * P:(i + 1) * P], tp,
                                         AF.Copy)
                else:
                    nc.vector.tensor_copy(xT[j][:, i * P:(i + 1) * P], tp)

        def ln_tile(i):
            ln_fin(i, ln_stats(i))

        # per-key bias projection; u = exp(bias) folds the softmax bias
        # into the ones-augmented V (exps become bias-free -> mergeable)
        def b_tile(i):
            # u = exp(xn@Wb); the +bb term is per-head constant and cancels
            # in the softmax normalization, so it is dropped entirely.
            ls = slice(i * P, (i + 1) * P)
            ps2 = psF.tile([P, HC], f32, tag="fsm", name=f"pb{i}", bufs=2)
            nc.tensor.matmul(ps2[:, 0:H], xT[0][:, ls], wb[:, 0, :],
                             start=True, stop=False)
            nc.tensor.matmul(ps2[:, 0:H], xT[1][:, ls], wb[:, 1, :],
                             start=False, stop=True)
            nc.scalar.activation(u[i][:], ps2[:, 0:H], AF.Exp)

        # v projection: vaug[.,h,c] = (v+bv)*u, vaug[.,h,C] = u
        def v_tile(i):
            ls = slice(i * P, (i + 1) * P)
            ps = psF.tile([P, HC], f32, tag="fsm", name=f"pv{i}", bufs=2)
            nc.tensor.matmul(ps[:], xT[0][:, ls], wv[:, 0, :],
                             start=True, stop=False)
            nc.tensor.matmul(ps[:], xT[1][:, ls], wv[:, 1, :],
                             start=False, stop=False)
            nc.tensor.matmul(ps[:], ones1[0:1, 0:P], bvr[:],
                             start=False, stop=True)
            nc.gpsimd.tensor_copy(
                vaug[i][:, :, C:C1].rearrange("p h one -> p (h one)"), u[i][:])
            ub = u[i][:].rearrange("p (h one) -> p h one", one=1)
            nc.vector.tensor_tensor(
                vaug[i][:, :, 0:C],
                ps[:].rearrange("p (h c) -> p h c", h=H),
                ub.broadcast_to([P, H, C]), op=ALU.mult)

        def g_tile(q):
            ls = slice(q * P, (q + 1) * P)
            ps = psF.tile([P, HC], f32, tag="fsm", name=f"pg{q}", bufs=2)
            nc.tensor.matmul(ps[:], xT[0][:, ls], wg[:, 0, :],
                             start=True, stop=False)
            nc.tensor.matmul(ps[:], xT[1][:, ls], wg[:, 1, :],
                             start=False, stop=False)
            nc.tensor.matmul(ps[:], ones1[0:1, 0:P], bgr[:],
                             start=False, stop=True)
            # ge = exp(-(y+bg)); gate = 1/(1+ge) folded into normalize
            nc.scalar.activation(ge[q][:], ps[:], AF.Exp, scale=-1.0)
            # prefetch g1 = 1 + ge per normalize band (bf16 4x on DVE)
            gv = ge[q][:].rearrange("p (h c) -> p h c", h=H)
            eng = nc.vector if q % 2 == 0 else nc.gpsimd
            eng.tensor_scalar(g1A[q][:], gv[:, 0:4, :], 1.0, None, op0=ALU.add)
            eng.tensor_scalar(g1B[q][:], gv[:, 4:7, :], 1.0, None, op0=ALU.add)
            eng.tensor_scalar(g17[q][:], gv[:, 7:8, :], 1.0, None, op0=ALU.add)

        def qk_m(j, m):
            ms = slice(512 * m, 512 * (m + 1))
            for wi, (w, bvec, dst) in enumerate(((wq, bq, qT), (wk, bk, kT))):
                ps = psF.tile([P, 512], f32, tag="fqk", name=f"p{j}{m}",
                              bufs=2)
                nc.tensor.matmul(ps[:], w[:, 0, j * P:(j + 1) * P],
                                 xT[0][:, ms], start=True, stop=False)
                nc.tensor.matmul(ps[:], w[:, 1, j * P:(j + 1) * P],
                                 xT[1][:, ms], start=False, stop=True)
                if wi == 0 or j == 1:
                    nc.vector.tensor_scalar(dst[j][:, ms], ps[:],
                                            bvec[:, j:j + 1], None, op0=ALU.add)
                else:
                    nc.scalar.activation(dst[j][:, ms], ps[:], AF.Identity,
                                         bias=bvec[:, j:j + 1])

        # ---- front pipeline: LN(0-3) -> qk m0 -> first wave exp ----
        sts = [ln_stats(i) for i in range(4)]
        for i in range(4):
            ln_fin(i, sts[i])
        qk_m(0, 0)

        # ============ Attention: 32 exp-groups of [128 x 2048] ============
        # group (h, kq, mh): S^T halves for ktiles 4kq..4kq+3, q-half mh.
        # One bias-free exp per group; AV partials accumulate back into the
        # group's own (already consumed) psum slot, then drain-add to SBUF.
        psW_cm = tc.tile_pool(name="psW", bufs=1, space="PSUM")
        psW = psW_cm.__enter__()

        WAVE = [(0, 0, 0), (1, 0, 0), (2, 0, 0)]
        groups = list(WAVE)
        groups += [(3, 0, 0)]
        groups += [(h, kq, mh) for h in range(4)
                   for (kq, mh) in ((0, 1), (1, 0), (1, 1))]
        groups += [(h, kq, mh) for h in range(4, 8)
                   for (kq, mh) in ((0, 0), (0, 1), (1, 0), (1, 1))]

        def emit_group(gi, pool, tag):
            h, kq, mh = groups[gi]
            jh, ph = h // 4, 32 * (h % 4)
            hp = slice(ph, ph + 32)
            sp = pool.tile([P, 2048], f32, tag=tag, name=f"sp{gi}")
            for b_ in range(4):
                kk = 4 * kq + b_
                nc.tensor.matmul(sp[:, b_ * 512:(b_ + 1) * 512],
                                 kT[jh][hp, kk * P:(kk + 1) * P],
                                 qT[jh][hp, mh * 512:(mh + 1) * 512],
                                 start=True, stop=True, tile_position=(ph, 0))
            eT = epool.tile([P, 2048], bf16, tag="e", name=f"e{gi}")
            nc.scalar.activation(eT[:], sp[:], AF.Exp)
            return (gi, sp, eT)

        def emit_AV(gi, sp, eT, drain_act=False):
            h, kq, mh = groups[gi]
            av = sp[:, 0:132].rearrange("p (qq c) -> p qq c", qq=4)
            for b_ in range(4):
                kk = 4 * kq + b_
                for ql in range(4):
                    nc.tensor.matmul(
                        av[:, ql, :],
                        eT[:, b_ * 512 + ql * P:b_ * 512 + (ql + 1) * P],
                        vaug[kk][:, h, :],
                        start=(b_ == 0 and ql == 0),
                        stop=(b_ == 3 and ql == 3))
            dst = att[:, mh * 4:(mh + 1) * 4, h, :]
            if kq == 0:
                if drain_act:
                    nc.scalar.activation(dst, av, AF.Copy)
                else:
                    nc.vector.tensor_copy(dst, av)
            else:
                nc.vector.tensor_tensor(dst, dst, av, op=ALU.add)

        # wave-1 (single-slot, serialized): fills the ACT idle window while
        # the rest of the front streams in
        pend = emit_group(0, psW, "w")
        emit_AV(*pend)
        for i in range(4):
            b_tile(i)
            v_tile(i)
        ln_tile(4)
        ln_tile(5)
        pend = emit_group(1, psW, "w")
        emit_AV(*pend)
        ln_tile(6)
        ln_tile(7)
        g_tile(0)
        g_tile(1)
        qk_m(0, 1)
        for i in range(4, NLT):
            b_tile(i)
            v_tile(i)
        pend = emit_group(2, psW, "w")
        sp_w2 = pend[1]
        emit_AV(*pend)
        qk_m(1, 0)
        qk_m(1, 1)
        # gates 4-7 ride the consumed wave slot (banks 1-2): their psums
        # are ready right after the wave, so the ge exps run during the
        # pool transition instead of gating it
        for q_ in range(2, NQT):
            ls = slice(q_ * P, (q_ + 1) * P)
            ps = sp_w2[:, 512 + 256 * (q_ - 2):768 + 256 * (q_ - 2)]
            nc.tensor.matmul(ps, xT[0][:, ls], wg[:, 0, :],
                             start=True, stop=False)
            nc.tensor.matmul(ps, xT[1][:, ls], wg[:, 1, :],
                             start=False, stop=False)
            nc.tensor.matmul(ps, ones1[0:1, 0:P], bgr[:],
                             start=False, stop=True)
            nc.scalar.activation(ge[q_][:], ps, AF.Exp, scale=-1.0)
            gv = ge[q_][:].rearrange("p (h c) -> p h c", h=H)
            eng = nc.vector if q_ % 2 == 0 else nc.gpsimd
            eng.tensor_scalar(g1A[q_][:], gv[:, 0:4, :], 1.0, None,
                             op0=ALU.add)
            eng.tensor_scalar(g1B[q_][:], gv[:, 4:7, :], 1.0, None,
                             op0=ALU.add)
            eng.tensor_scalar(g17[q_][:], gv[:, 7:8, :], 1.0, None,
                             op0=ALU.add)
        pend = None

        psW_cm.__exit__(None, None, None)
        psF_cm.__exit__(None, None, None)

        # ---- steady stream ----
        psS_cm = tc.tile_pool(name="psS", bufs=2, space="PSUM")
        psS = psS_cm.__enter__()

        tq0_done = 0
        norm_q = []
        prev_sp = None
        for gi in range(len(WAVE), len(groups)):
            nxt = emit_group(gi, psS, "s")
            if pend is not None:
                pgi = pend[0]
                prev_sp = pend[1]
                emit_AV(*pend)
                # normalize bands, 2 chains per group on Pool so DVE keeps
                # the slot-freeing psum drains moving
                for _ in range(2):
                    if norm_q and norm_q[0][0] <= pgi:
                        _, band, q = norm_q.pop(0)
                        g1t = g1A[q] if band == 0 else g1B[q]
                        _norm(nc, mybir, npool, att, g1t, ag, q,
                              0 if band == 0 else 4, 4 if band == 0 else 3,
                              engine=1)
                if groups[pgi] == (3, 1, 1):
                    norm_q += [(pgi, 0, q) for q in range(NQT)]
                if groups[pgi] == (6, 1, 1):
                    norm_q += [(pgi, 1, q) for q in range(NQT)]
            pend = nxt
            # piggyback jh0 ag-transposes into consumed h4/h5 slots (bank 1
            # is free after that group's exp; bank 0 holds the AV partial)
            h = groups[gi][0]
            if h in (4, 5) and prev_sp is not None and tq0_done < NQT:
                q = tq0_done
                tq_ap = prev_sp[:, 576:640].bitcast(bf16)
                nc.tensor.transpose(tq_ap, ag[q][:, 0:P], identb[:])
                nc.vector.tensor_copy(agT[0][:, q * P:(q + 1) * P], tq_ap)
                tq0_done += 1
            if gi == len(groups) - 1:
                # ---- tail, fully slot-riding ----
                # qq 0-3 chains use the (7,1,0) slot (prev_sp) while the
                # final exp runs; qq 4-7 use the (7,1,1) slot (pend) after
                # its exp+AV.
                def wo_chain(q, sp_t):
                    ls = slice(q * P, (q + 1) * P)
                    qm = q % 4
                    po = sp_t[:, 1024 + 256 * qm:1280 + 256 * qm]
                    nc.tensor.matmul(po, agT[0][:, ls], wo[:, 0, :],
                                     start=True, stop=False)
                    nc.tensor.matmul(po, agT[1][:, ls], wo[:, 1, :],
                                     start=False, stop=False)
                    nc.tensor.matmul(po, ones1[0:1, 0:P], bor[:],
                                     start=False, stop=True)
                    o = opool.tile([P, F], f32, tag="oo", name=f"oE{q}")
                    if q % 2 == 0:
                        nc.scalar.activation(o[:], po, AF.Copy)
                    else:
                        nc.vector.tensor_copy(o[:], po)
                    (nc.sync if q % 2 == 0 else nc.gpsimd).dma_start(
                        out_e.ap()[ls, :], o[:])

                # qq0-3 chains run during the final exp
                for q in range(4):
                    _norm(nc, mybir, npool, att, g17[q], ag, q, 7, 1,
                          engine=q % 2)
                for q in range(4):
                    ls = slice(q * P, (q + 1) * P)
                    tq_ap = prev_sp[:, 512 + 64 * q:576 + 64 * q].bitcast(bf16)
                    nc.tensor.transpose(tq_ap, ag[q][:, P:2 * P], identb[:])
                    if q % 2 == 0:
                        nc.scalar.activation(agT[1][:, ls], tq_ap, AF.Copy)
                    else:
                        nc.vector.tensor_copy(agT[1][:, ls], tq_ap)
                # final AV + drain the moment the last exp finishes
                sp_last = pend[1]
                emit_AV(*pend)
                pend = None
                for q in range(4, NQT):
                    _norm(nc, mybir, npool, att, g17[q], ag, q, 7, 1,
                          engine=q % 2)
                for q in range(4):
                    wo_chain(q, prev_sp)
                for q in range(4, NQT):
                    ls = slice(q * P, (q + 1) * P)
                    qm = q - 4
                    tq_ap = sp_last[:, 512 + 64 * qm:576 + 64 * qm].bitcast(bf16)
                    nc.tensor.transpose(tq_ap, ag[q][:, P:2 * P], identb[:])
                    if q % 2 == 0:
                        nc.scalar.activation(agT[1][:, ls], tq_ap, AF.Copy)
                    else:
                        nc.vector.tensor_copy(agT[1][:, ls], tq_ap)
                for q in range(4, NQT):
                    wo_chain(q, sp_last)
        psS_cm.__exit__(None, None, None)

    # Restrict Exp/Ln/Square to the combined table so one load serves all.
    import concourse.bacc as bacc_mod
    orig_gat = bacc_mod.get_activation_tables
    AFt = mybir.ActivationFunctionType

    def gat_combined(arch):
        t = orig_gat(arch)
        out = {}
        drop = {AFt.Exp, AFt.Ln, AFt.Square}
        for name, funcs in t.items():
            if name == "natural_log_exp_and_others":
                out[name] = funcs
            else:
                out[name] = funcs - drop
        return out

    bacc_mod.get_activation_tables = gat_combined
    try:
        nc.compile()
    finally:
        bacc_mod.get_activation_tables = orig_gat
    return nc


def _norm(nc, mybir, npool, att, g1t, ag, q, h0, nh, engine):
    """ag[q][:, h0*32:(h0+nh)*32] = N * 1/(D*(1+ge)) for heads h0..h0+nh-1."""
    ALU = mybir.AluOpType
    f32 = mybir.dt.float32
    hs = slice(h0 * C, (h0 + nh) * C)
    dg = npool.tile([P, nh, C], f32, tag=f"dg_{h0}", name=f"dg_{h0}_{q}")
    rc = npool.tile([P, nh, C], f32, tag=f"rc_{h0}", name=f"rc_{h0}_{q}")
    eng = nc.vector if engine == 0 else nc.gpsimd
    dsrc = att[:, q, h0:h0 + nh, C:C1].broadcast_to([P, nh, C])
    eng.tensor_tensor(dg[:], dsrc, g1t[:], op=ALU.mult)
    nc.vector.reciprocal(rc[:], dg[:])
    nsrc = att[:, q, h0:h0 + nh, 0:C]
    dst = ag[q][:, hs].rearrange("p (h c) -> p h c", h=nh)
    eng.tensor_tensor(dst, nsrc, rc[:], op=ALU.mult)


def _norm_pool(nc, mybir, npool, att, g1t, ag, q):
    """h7 normalize with multiplies on Pool (keeps the tail DVE light)."""
    ALU = mybir.AluOpType
    f32 = mybir.dt.float32
    hs = slice(7 * C, 8 * C)
    dg = npool.tile([P, 1, C], f32, tag="dg_t", name=f"dgt{q}")
    rc = npool.tile([P, 1, C], f32, tag="rc_t", name=f"rct{q}")
    dsrc = att[:, q, 7:8, C:C1].broadcast_to([P, 1, C])
    nc.gpsimd.tensor_tensor(dg[:], dsrc, g1t[:], op=ALU.mult)
    nc.vector.reciprocal(rc[:], dg[:])
    nsrc = att[:, q, 7:8, 0:C]
    dst = ag[q][:, hs].rearrange("p (h c) -> p h c", h=1)
    nc.gpsimd.tensor_tensor(dst, nsrc, rc[:], op=ALU.mult)


def _prep_inputs(features, ln_g, ln_b, Wq, bq, Wk, bk, Wv, bv, Wb, bb,
                 Wg, bg, Wo, bo):
    f32 = np.float32
    sq = f32(1.0 / np.sqrt(C))
    g_ = np.asarray(ln_g, f32)[:, None]
    b_ = np.asarray(ln_b, f32)

    def wsplit(W, n):
        return np.ascontiguousarray(
            np.asarray(W, f32).reshape(NFC, P, n).transpose(1, 0, 2))

    def bsplit(b):
        return np.ascontiguousarray(np.asarray(b, f32).reshape(NFC, P).T)

    Wq_ = np.asarray(Wq, f32) * g_ * sq
    bq_ = (b_ @ (np.asarray(Wq, f32) * sq) + np.asarray(bq, f32) * sq)
    Wk_ = np.asarray(Wk, f32) * g_
    bk_ = b_ @ np.asarray(Wk, f32) + np.asarray(bk, f32)
    Wv_ = np.asarray(Wv, f32) * g_
    bv_ = b_ @ np.asarray(Wv, f32) + np.asarray(bv, f32)
    Wg_ = np.asarray(Wg, f32) * g_
    bg_ = b_ @ np.asarray(Wg, f32) + np.asarray(bg, f32)
    Wb_ = np.asarray(Wb, f32) * g_
    bb_ = b_ @ np.asarray(Wb, f32) + np.asarray(bb, f32)

    common = {
        "wq": wsplit(Wq_, HC),
        "wk": wsplit(Wk_, HC),
        "wv": wsplit(Wv_, HC),
        "wg": wsplit(Wg_, HC),
        "wb": wsplit(Wb_, H),
        "wo": wsplit(Wo, F),
        "bq_t": bsplit(bq_),
        "bk_t": bsplit(bk_),
        "bv_r": np.ascontiguousarray(bv_[None, :]),
        "bg_r": np.ascontiguousarray(bg_[None, :]),
        "bo_r": np.ascontiguousarray(np.asarray(bo, f32)[None, :]),
        "bb_b": np.ascontiguousarray(np.tile(bb_, (P, 1))),
        "ident": np.eye(P, dtype=f32),
        "ones1": np.ones((1, P), f32),
    }
    feats = np.asarray(features, f32)
    in_maps = []
    for c_ in range(N_CORES):
        m = dict(common)
        m["feat"] = np.ascontiguousarray(feats[:, c_, :])
        in_maps.append(m)
    return in_maps


def kernel(**inputs):
    from concourse.bass_utils import run_bass_kernel_spmd

    if "nc" not in _COMPILED:
        _COMPILED["nc"] = _build()
    nc = _COMPILED["nc"]
    in_maps = _prep_inputs(**inputs)
    res = run_bass_kernel_spmd(nc, in_maps, list(range(N_CORES)))
    out = np.stack([res.results[c_]["out"] for c_ in range(N_CORES)], axis=1)
    return np.ascontiguousarray(out.astype(np.float32))


if __name__ == "__main__":
    rng = np.random.default_rng(0)
    ins = {
        "features": rng.standard_normal((L, B, F), dtype=np.float32),
        "ln_g": np.ones(F, np.float32), "ln_b": np.zeros(F, np.float32),
        "Wq": rng.standard_normal((F, HC), dtype=np.float32) * 0.02,
        "bq": np.zeros(HC, np.float32),
        "Wk": rng.standard_normal((F, HC), dtype=np.float32) * 0.02,
        "bk": np.zeros(HC, np.float32),
        "Wv": rng.standard_normal((F, HC), dtype=np.float32) * 0.02,
        "bv": np.zeros(HC, np.float32),
        "Wb": rng.standard_normal((F, H), dtype=np.float32) * 0.02,
        "bb": np.zeros(H, np.float32),
        "Wg": rng.standard_normal((F, HC), dtype=np.float32) * 0.02,
        "bg": np.zeros(HC, np.float32),
        "Wo": rng.standard_normal((HC, F), dtype=np.float32) * 0.02,
        "bo": np.zeros(F, np.float32),
    }
    print(kernel(**ins).shape)


# revision 37
# speedup vs baseline: 1.0002x; 1.0002x over previous
"""Trainium2 Bass kernel for nn_PairwiseAttentionTerminal.

Reference computation (L=1024, B=8, F=256, H=8, C=32):
    x = layernorm(features)                       # (L, B, F)
    q,k,v = x@Wq+bq, x@Wk+bk, x@Wv+bv             # (L, B, H, C)
    bias  = x@Wb+bb                               # (L, B, H) per-key bias
    gate  = sigmoid(x@Wg+bg)                      # (L, B, H, C)
    S     = einsum('qbhc,kbhc->qbkh', q, k)/sqrt(C) + bias[None]
    attn  = softmax_k(S) @ v                      # (L, B, H, C)
    out   = (attn*gate) @ Wo + bo                 # (L, B, F)

Sharding: batch B=8 -> one batch element per NeuronCore (8 cores), weights
replicated, no collectives.  Host shards/gathers around one SPMD NEFF.

Per-core engine plan (cost-model driven):
  - ACT is the bottleneck: 64 softmax exps of [128k x 1024q] from PSUM
    (per-key bias = per-partition ACT bias operand).  Everything else is
    arranged to hide under that stream.
  - S^T[k,q] per (head, ktile): 32-contraction f32r matmuls (2 x 512 free).
  - AV restructured as out[q, c]: stationary = eT [128k x 128q] slice (bf16),
    moving = ones-augmented V [128k x 33] (bf16) -> 33-cycle matmuls into a
    single-bank accumulator [128, 8qq, 33]; denominator rides along as col 32.
    AV emission is software-pipelined one (h,kk) step behind the S/exp stream
    so the in-order PE queue never blocks on the current exp.
  - Normalize in q-major layout: DVE reciprocal of D*(1+e^-y) fuses the
    sigmoid gate division; no DRAM broadcast roundtrip.  Heads 0-3 normalize
    under the h4-6 exp stream, 4-6 under h7, only h7 in the tail.
  - gate/v/output biases are rank-1 matmuls (ones[1,128] x bias_row) chained
    into the projection accumulation.
  - ag (gated attn, q-major bf16) -> PE-transposed (bf16 identity, 1 cyc/row)
    -> Wo projection per qtile, pipelined drain+DMA tail.
  - PE heater matmuls at t=0 ramp the PE clock (p-state) before the real
    front (LN -> transpose -> q/k projections) hits it.
"""

import numpy as np
from contextlib import ExitStack

L, B, F, H, C = 1024, 8, 256, 8, 32
HC = H * C
C1 = C + 1
EPS = 1e-5
N_CORES = 8
P = 128
NLT = L // P   # 8 L-tiles (== qtiles == ktiles)
NFC = F // P   # 2 F-chunks
NQT = L // P   # 8 q-tiles

_COMPILED = {}


def _build():
    import concourse.bacc as bacc
    import concourse.mybir as mybir
    import concourse.tile as tile

    f32 = mybir.dt.float32
    f32r = mybir.dt.float32r
    bf16 = mybir.dt.bfloat16
    AF = mybir.ActivationFunctionType
    ALU = mybir.AluOpType

    nc = bacc.Bacc("TRN2", target_bir_lowering=False)

    # ---- DRAM I/O (per-core) ----
    feat_e = nc.dram_tensor("feat", [L, F], f32, kind="ExternalInput")
    wq_e = nc.dram_tensor("wq", [P, NFC, HC], f32r, kind="ExternalInput")
    wk_e = nc.dram_tensor("wk", [P, NFC, HC], f32r, kind="ExternalInput")
    wv_e = nc.dram_tensor("wv", [P, NFC, HC], f32r, kind="ExternalInput")
    wg_e = nc.dram_tensor("wg", [P, NFC, HC], f32r, kind="ExternalInput")
    wb_e = nc.dram_tensor("wb", [P, NFC, H], f32r, kind="ExternalInput")
    wo_e = nc.dram_tensor("wo", [P, NFC, F], f32r, kind="ExternalInput")
    bq_e = nc.dram_tensor("bq_t", [P, NFC], f32, kind="ExternalInput")
    bk_e = nc.dram_tensor("bk_t", [P, NFC], f32, kind="ExternalInput")
    bv_e = nc.dram_tensor("bv_r", [1, HC], f32r, kind="ExternalInput")
    bg_e = nc.dram_tensor("bg_r", [1, HC], f32r, kind="ExternalInput")
    bo_e = nc.dram_tensor("bo_r", [1, F], f32r, kind="ExternalInput")
    bb_e = nc.dram_tensor("bb_b", [P, H], f32, kind="ExternalInput")
    id_e = nc.dram_tensor("ident", [P, P], f32r, kind="ExternalInput")
    ones_e = nc.dram_tensor("ones1", [1, P], f32r, kind="ExternalInput")
    out_e = nc.dram_tensor("out", [L, F], f32, kind="ExternalOutput")

    with tile.TileContext(nc) as tc, ExitStack() as ctx:
        const = ctx.enter_context(tc.tile_pool(name="const", bufs=1))
        main = ctx.enter_context(tc.tile_pool(name="main", bufs=1))
        work = ctx.enter_context(tc.tile_pool(name="work", bufs=4))
        epool = ctx.enter_context(tc.tile_pool(name="epool", bufs=4))
        npool = ctx.enter_context(tc.tile_pool(name="npool", bufs=8))
        opool = ctx.enter_context(tc.tile_pool(name="opool", bufs=4))

        # ---- t=0: heater fuel + ACT table prewarm source ----
        ones512 = const.tile([P, 512], f32, name="ones512")
        nc.vector.memset(ones512[:], 1.0)

        # ---- input DMAs, ordered by first use ----
        ft = [const.tile([P, F], f32, name=f"ft{i}") for i in range(NLT)]
        for i in range(4):
            nc.sync.dma_start(ft[i][:], feat_e.ap()[i * P:(i + 1) * P, :])

        def load(eng, name, ext, shape, dt_=f32):
            t = const.tile(shape, dt_, name=name)
            eng.dma_start(t[:], ext.ap())
            return t

        wq = load(nc.sync, "wq_s", wq_e, [P, NFC, HC], f32r)
        wk = load(nc.sync, "wk_s", wk_e, [P, NFC, HC], f32r)
        bq = load(nc.sync, "bq_s", bq_e, [P, NFC])
        bk = load(nc.sync, "bk_s", bk_e, [P, NFC])
        bvr = load(nc.sync, "bv_s", bv_e, [1, HC], f32r)
        for i in range(4, NLT):
            nc.gpsimd.dma_start(ft[i][:], feat_e.ap()[i * P:(i + 1) * P, :])
        ident = load(nc.gpsimd, "id_s", id_e, [P, P], f32r)
        wb = load(nc.gpsimd, "wb_s", wb_e, [P, NFC, H], f32r)
        ones1 = load(nc.gpsimd, "ones1_s", ones_e, [1, P], f32r)
        wv = load(nc.gpsimd, "wv_s", wv_e, [P, NFC, HC], f32r)
        wg = load(nc.gpsimd, "wg_s", wg_e, [P, NFC, HC], f32r)
        bgr = load(nc.gpsimd, "bg_s", bg_e, [1, HC], f32r)
        wo = load(nc.sync, "wo_s", wo_e, [P, NFC, F], f32r)
        bor = load(nc.sync, "bo_s", bo_e, [1, F], f32r)

        # ACT table prewarm: one Ln on the memset-ones tile loads the
        # combined ln/exp table before the front needs it.
        scr0 = const.tile([P, 2], f32, name="scr0")
        nc.scalar.activation(scr0[:, 0:1], ones512[:, 0:1], AF.Ln)

        epst = const.tile([P, 1], f32, name="epst")
        nc.vector.memset(epst[:], EPS)

        # bf16 identity for the ag transposes (1 cyc/row vs 1.5 for f32r);
        # copied on Pool so it stays off the DVE LN stream
        identb = const.tile([P, P], bf16, name="identb")
        nc.gpsimd.tensor_copy(identb[:], ident[:])

        # ---- persistent tiles ----
        xT = [main.tile([P, L], f32r, name=f"xT{j}") for j in range(NFC)]
        qT = [main.tile([P, L], f32r, name=f"qT{j}") for j in range(NFC)]
        kT = [main.tile([P, L], f32r, name=f"kT{j}") for j in range(NFC)]
        vaug = [main.tile([P, H, C1], bf16, name=f"vaug{i}") for i in range(NLT)]
        u = [main.tile([P, H], f32, name=f"u{i}") for i in range(NLT)]
        ge = [main.tile([P, HC], bf16, name=f"ge{q}") for q in range(NQT)]
        g1A = [main.tile([P, 4, C], bf16, name=f"g1A{q}") for q in range(NQT)]
        g1B = [main.tile([P, 3, C], bf16, name=f"g1B{q}") for q in range(NQT)]
        g17 = [main.tile([P, 1, C], bf16, name=f"g17{q}") for q in range(NQT)]
        att = main.tile([P, NQT, H, C1], f32, name="att")
        ag = [main.tile([P, HC], bf16, name=f"ag{q}") for q in range(NQT)]
        agT = [main.tile([P, L], f32r, name=f"agT{j}") for j in range(NFC)]

        # ================= Front phase =================
        # psF: ftp 1 bank + fqk 1 + fsm 2 = 4 banks; psW (wave-1) 4 banks.
        psF_cm = tc.tile_pool(name="psF", bufs=1, space="PSUM")
        psF = psF_cm.__enter__()

        def heat(n):
            # heaters ride the fqk slot (same shape as qk psum tiles)
            for _ in range(n):
                hp_ = psF.tile([P, 512], f32, tag="fqk", name="heat", bufs=2)
                nc.tensor.matmul(hp_[:], ones512[:, 0:P].bitcast(f32r),
                                 ones512[:].bitcast(f32r),
                                 start=True, stop=True)

        heat(6)

        # LN per L-tile: stats (DVE-paced) split from finish so the DVE
        # stream never blocks on the cross-engine rstd wait
        def ln_stats(i):
            st = work.tile([P, 8], f32, tag="st", bufs=8)
            nc.vector.bn_stats(st[:, 0:6], ft[i][:])
            nc.vector.bn_aggr(st[:, 6:8], st[:, 0:6])
            # rstd = exp(-0.5*ln(var+eps)) (free: scalar ops cost 0)
            nc.scalar.activation(st[:, 3:4], st[:, 7:8], AF.Ln, bias=epst[:])
            nc.scalar.activation(st[:, 4:5], st[:, 3:4], AF.Exp, scale=-0.5)
            return st

        def ln_fin(i, st):
            xn = work.tile([P, F], f32r, tag="xn")
            nc.vector.tensor_scalar(xn[:], ft[i][:], st[:, 6:7], st[:, 4:5],
                                    op0=ALU.subtract, op1=ALU.mult)
            for j in range(NFC):
                tpw = psF.tile([P, 512], f32r, tag="fqk", name=f"tp{i}_{j}",
                               bufs=2)
                tp = tpw[:, 0:P]
                nc.tensor.transpose(tp, xn[:, j * P:(j + 1) * P], ident[:])
                # alternate drains ACT/DVE (ACT idle during the front)
                if (2 * i + j) % 2 == 0:
                    nc.scalar.activation(xT[j][:, i * P:(i + 1) * P], tp,
                                         AF.Copy)
                else:
                    nc.vector.tensor_copy(xT[j][:, i * P:(i + 1) * P], tp)

        def ln_tile(i):
            ln_fin(i, ln_stats(i))

        # per-key bias projection; u = exp(bias) folds the softmax bias
        # into the ones-augmented V (exps become bias-free -> mergeable)
        def b_tile(i):
            # u = exp(xn@Wb); the +bb term is per-head constant and cancels
            # in the softmax normalization, so it is dropped entirely.
            ls = slice(i * P, (i + 1) * P)
            ps2 = psF.tile([P, HC], f32, tag="fsm", name=f"pb{i}", bufs=2)
            nc.tensor.matmul(ps2[:, 0:H], xT[0][:, ls], wb[:, 0, :],
                             start=True, stop=False)
            nc.tensor.matmul(ps2[:, 0:H], xT[1][:, ls], wb[:, 1, :],
                             start=False, stop=True)
            nc.scalar.activation(u[i][:], ps2[:, 0:H], AF.Exp)

        # v projection: vaug[.,h,c] = (v+bv)*u, vaug[.,h,C] = u
        def v_tile(i):
            ls = slice(i * P, (i + 1) * P)
            ps = psF.tile([P, HC], f32, tag="fsm", name=f"pv{i}", bufs=2)
            nc.tensor.matmul(ps[:], xT[0][:, ls], wv[:, 0, :],
                             start=True, stop=False)
            nc.tensor.matmul(ps[:], xT[1][:, ls], wv[:, 1, :],
                             start=False, stop=False)
            nc.tensor.matmul(ps[:], ones1[0:1, 0:P], bvr[:],
                             start=False, stop=True)
            nc.gpsimd.tensor_copy(
                vaug[i][:, :, C:C1].rearrange("p h one -> p (h one)"), u[i][:])
            ub = u[i][:].rearrange("p (h one) -> p h one", one=1)
            nc.vector.tensor_tensor(
                vaug[i][:, :, 0:C],
                ps[:].rearrange("p (h c) -> p h c", h=H),
                ub.broadcast_to([P, H, C]), op=ALU.mult)

        def g_tile(q):
            ls = slice(q * P, (q + 1) * P)
            ps = psF.tile([P, HC], f32, tag="fsm", name=f"pg{q}", bufs=2)
            nc.tensor.matmul(ps[:], xT[0][:, ls], wg[:, 0, :],
                             start=True, stop=False)
            nc.tensor.matmul(ps[:], xT[1][:, ls], wg[:, 1, :],
                             start=False, stop=False)
            nc.tensor.matmul(ps[:], ones1[0:1, 0:P], bgr[:],
                             start=False, stop=True)
            # ge = exp(-(y+bg)); gate = 1/(1+ge) folded into normalize
            nc.scalar.activation(ge[q][:], ps[:], AF.Exp, scale=-1.0)
            # prefetch g1 = 1 + ge per normalize band (bf16 4x on DVE)
            gv = ge[q][:].rearrange("p (h c) -> p h c", h=H)
            eng = nc.vector if q % 2 == 0 else nc.gpsimd
            eng.tensor_scalar(g1A[q][:], gv[:, 0:4, :], 1.0, None, op0=ALU.add)
            eng.tensor_scalar(g1B[q][:], gv[:, 4:7, :], 1.0, None, op0=ALU.add)
            eng.tensor_scalar(g17[q][:], gv[:, 7:8, :], 1.0, None, op0=ALU.add)

        def qk_m(j, m):
            ms = slice(512 * m, 512 * (m + 1))
            for wi, (w, bvec, dst) in enumerate(((wq, bq, qT), (wk, bk, kT))):
                ps = psF.tile([P, 512], f32, tag="fqk", name=f"p{j}{m}",
                              bufs=2)
                nc.tensor.matmul(ps[:], w[:, 0, j * P:(j + 1) * P],
                                 xT[0][:, ms], start=True, stop=False)
                nc.tensor.matmul(ps[:], w[:, 1, j * P:(j + 1) * P],
                                 xT[1][:, ms], start=False, stop=True)
                if j == 1 or wi == 1:
                    # ACT Identity+bias: keeps chunk-1 drains off the
                    # DVE backlog so psF retires early
                    nc.scalar.activation(dst[j][:, ms], ps[:], AF.Identity,
                                         bias=bvec[:, j:j + 1])
                else:
                    nc.vector.tensor_scalar(dst[j][:, ms], ps[:],
                                            bvec[:, j:j + 1], None, op0=ALU.add)

        # ---- front pipeline: LN(0-3) -> qk m0 -> first wave exp ----
        sts = [ln_stats(i) for i in range(4)]
        for i in range(4):
            ln_fin(i, sts[i])
        qk_m(0, 0)

        # ============ Attention: 32 exp-groups of [128 x 2048] ============
        # group (h, kq, mh): S^T halves for ktiles 4kq..4kq+3, q-half mh.
        # One bias-free exp per group; AV partials accumulate back into the
        # group's own (already consumed) psum slot, then drain-add to SBUF.
        psW_cm = tc.tile_pool(name="psW", bufs=1, space="PSUM")
        psW = psW_cm.__enter__()

        WAVE = [(0, 0, 0), (1, 0, 0), (2, 0, 0)]
        groups = list(WAVE)
        groups += [(3, 0, 0)]
        groups += [(h, kq, mh) for h in range(4)
                   for (kq, mh) in ((0, 1), (1, 0), (1, 1))]
        groups += [(h, kq, mh) for h in range(4, 8)
                   for (kq, mh) in ((0, 0), (0, 1), (1, 0), (1, 1))]

        def emit_group(gi, pool, tag):
            h, kq, mh = groups[gi]
            jh, ph = h // 4, 32 * (h % 4)
            hp = slice(ph, ph + 32)
            sp = pool.tile([P, 2048], f32, tag=tag, name=f"sp{gi}")
            for b_ in range(4):
                kk = 4 * kq + b_
                nc.tensor.matmul(sp[:, b_ * 512:(b_ + 1) * 512],
                                 kT[jh][hp, kk * P:(kk + 1) * P],
                                 qT[jh][hp, mh * 512:(mh + 1) * 512],
                                 start=True, stop=True, tile_position=(ph, 0))
            eT = epool.tile([P, 2048], bf16, tag="e", name=f"e{gi}")
            nc.scalar.activation(eT[:], sp[:], AF.Exp)
            return (gi, sp, eT)

        def emit_AV(gi, sp, eT, drain_act=False):
            h, kq, mh = groups[gi]
            av = sp[:, 0:132].rearrange("p (qq c) -> p qq c", qq=4)
            for b_ in range(4):
                kk = 4 * kq + b_
                for ql in range(4):
                    nc.tensor.matmul(
                        av[:, ql, :],
                        eT[:, b_ * 512 + ql * P:b_ * 512 + (ql + 1) * P],
                        vaug[kk][:, h, :],
                        start=(b_ == 0 and ql == 0),
                        stop=(b_ == 3 and ql == 3))
            dst = att[:, mh * 4:(mh + 1) * 4, h, :]
            if kq == 0:
                if drain_act:
                    nc.scalar.activation(dst, av, AF.Copy)
                else:
                    nc.vector.tensor_copy(dst, av)
            else:
                nc.vector.tensor_tensor(dst, dst, av, op=ALU.add)

        # wave-1 (single-slot, serialized): fills the ACT idle window while
        # the rest of the front streams in
        pend = emit_group(0, psW, "w")
        emit_AV(*pend)
        for i in range(4):
            b_tile(i)
            v_tile(i)
        ln_tile(4)
        ln_tile(5)
        pend = emit_group(1, psW, "w")
        emit_AV(*pend)
        ln_tile(6)
        ln_tile(7)
        g_tile(0)
        g_tile(1)
        qk_m(0, 1)
        for i in range(4, NLT):
            b_tile(i)
            v_tile(i)
        g_tile(2)
        g_tile(3)
        pend = emit_group(2, psW, "w")
        sp_w2 = pend[1]
        emit_AV(*pend)
        qk_m(1, 0)
        qk_m(1, 1)
        # gates 4-7 ride the consumed wave slot (banks 1-2): their psums
        # are ready right after the wave, so the ge exps run during the
        # pool transition instead of gating it
        for q_ in range(4, NQT):
            ls = slice(q_ * P, (q_ + 1) * P)
            ps = sp_w2[:, 512 + 256 * (q_ - 4):768 + 256 * (q_ - 4)]
            nc.tensor.matmul(ps, xT[0][:, ls], wg[:, 0, :],
                             start=True, stop=False)
            nc.tensor.matmul(ps, xT[1][:, ls], wg[:, 1, :],
                             start=False, stop=False)
            nc.tensor.matmul(ps, ones1[0:1, 0:P], bgr[:],
                             start=False, stop=True)
            nc.scalar.activation(ge[q_][:], ps, AF.Exp, scale=-1.0)
            gv = ge[q_][:].rearrange("p (h c) -> p h c", h=H)
            eng = nc.vector if q_ % 2 == 0 else nc.gpsimd
            eng.tensor_scalar(g1A[q_][:], gv[:, 0:4, :], 1.0, None,
                             op0=ALU.add)
            eng.tensor_scalar(g1B[q_][:], gv[:, 4:7, :], 1.0, None,
                             op0=ALU.add)
            eng.tensor_scalar(g17[q_][:], gv[:, 7:8, :], 1.0, None,
                             op0=ALU.add)
        pend = None

        psW_cm.__exit__(None, None, None)
        psF_cm.__exit__(None, None, None)

        # ---- steady stream ----
        psS_cm = tc.tile_pool(name="psS", bufs=2, space="PSUM")
        psS = psS_cm.__enter__()

        tq0_done = 0
        norm_q = []
        prev_sp = None
        for gi in range(len(WAVE), len(groups)):
            nxt = emit_group(gi, psS, "s")
            if pend is not None:
                pgi = pend[0]
                prev_sp = pend[1]
                emit_AV(*pend)
                # normalize bands, 2 chains per group on Pool so DVE keeps
                # the slot-freeing psum drains moving
                for _ in range(2):
                    if norm_q and norm_q[0][0] <= pgi:
                        _, band, q = norm_q.pop(0)
                        g1t = g1A[q] if band == 0 else g1B[q]
                        _norm(nc, mybir, npool, att, g1t, ag, q,
                              0 if band == 0 else 4, 4 if band == 0 else 3,
                              engine=1)
                if groups[pgi] == (3, 1, 1):
                    norm_q += [(pgi, 0, q) for q in range(NQT)]
                if groups[pgi] == (6, 1, 1):
                    norm_q += [(pgi, 1, q) for q in range(NQT)]
            pend = nxt
            # piggyback jh0 ag-transposes into consumed h4/h5 slots (bank 1
            # is free after that group's exp; bank 0 holds the AV partial)
            h = groups[gi][0]
            if h in (4, 5) and prev_sp is not None and tq0_done < NQT:
                q = tq0_done
                tq_ap = prev_sp[:, 576:640].bitcast(bf16)
                nc.tensor.transpose(tq_ap, ag[q][:, 0:P], identb[:])
                nc.vector.tensor_copy(agT[0][:, q * P:(q + 1) * P], tq_ap)
                tq0_done += 1
            if gi == len(groups) - 1:
                # ---- tail, fully slot-riding ----
                # qq 0-3 chains use the (7,1,0) slot (prev_sp) while the
                # final exp runs; qq 4-7 use the (7,1,1) slot (pend) after
                # its exp+AV.
                def wo_chain(q, sp_t):
                    ls = slice(q * P, (q + 1) * P)
                    qm = q % 4
                    po = sp_t[:, 1024 + 256 * qm:1280 + 256 * qm]
                    nc.tensor.matmul(po, agT[0][:, ls], wo[:, 0, :],
                                     start=True, stop=False)
                    nc.tensor.matmul(po, agT[1][:, ls], wo[:, 1, :],
                                     start=False, stop=False)
                    nc.tensor.matmul(po, ones1[0:1, 0:P], bor[:],
                                     start=False, stop=True)
                    o = opool.tile([P, F], f32, tag="oo", name=f"oE{q}")
                    if q % 2 == 0:
                        nc.scalar.activation(o[:], po, AF.Copy)
                    else:
                        nc.vector.tensor_copy(o[:], po)
                    (nc.sync if q % 2 == 0 else nc.gpsimd).dma_start(
                        out_e.ap()[ls, :], o[:])

                # qq0-3 chains run during the final exp
                for q in range(4):
                    _norm(nc, mybir, npool, att, g17[q], ag, q, 7, 1,
                          engine=q % 2)
                for q in range(4):
                    ls = slice(q * P, (q + 1) * P)
                    tq_ap = prev_sp[:, 512 + 64 * q:576 + 64 * q].bitcast(bf16)
                    nc.tensor.transpose(tq_ap, ag[q][:, P:2 * P], identb[:])
                    if q % 2 == 0:
                        nc.scalar.activation(agT[1][:, ls], tq_ap, AF.Copy)
                    else:
                        nc.vector.tensor_copy(agT[1][:, ls], tq_ap)
                # final AV + drain the moment the last exp finishes
                sp_last = pend[1]
                emit_AV(*pend)
                pend = None
                for q in range(4, NQT):
                    _norm(nc, mybir, npool, att, g17[q], ag, q, 7, 1,
                          engine=q % 2)
                for q in range(4):
                    wo_chain(q, prev_sp)
                for q in range(4, NQT):
                    ls = slice(q * P, (q + 1) * P)
                    qm = q - 4
                    tq_ap = sp_last[:, 512 + 64 * qm:576 + 64 * qm].bitcast(bf16)
                    nc.tensor.transpose(tq_ap, ag[q][:, P:2 * P], identb[:])
                    if q % 2 == 0:
                        nc.scalar.activation(agT[1][:, ls], tq_ap, AF.Copy)
                    else:
                        nc.vector.tensor_copy(agT[1][:, ls], tq_ap)
                for q in range(4, NQT):
                    wo_chain(q, sp_last)
        psS_cm.__exit__(None, None, None)

    # Restrict Exp/Ln/Square to the combined table so one load serves all.
    import concourse.bacc as bacc_mod
    orig_gat = bacc_mod.get_activation_tables
    AFt = mybir.ActivationFunctionType

    def gat_combined(arch):
        t = orig_gat(arch)
        out = {}
        drop = {AFt.Exp, AFt.Ln, AFt.Square}
        for name, funcs in t.items():
            if name == "natural_log_exp_and_others":
                out[name] = funcs
            else:
                out[name] = funcs - drop
        return out

    bacc_mod.get_activation_tables = gat_combined
    try:
        nc.compile()
    finally:
        bacc_mod.get_activation_tables = orig_gat
    return nc


def _norm(nc, mybir, npool, att, g1t, ag, q, h0, nh, engine):
    """ag[q][:, h0*32:(h0+nh)*32] = N * 1/(D*(1+ge)) for heads h0..h0+nh-1."""
    ALU = mybir.AluOpType
    f32 = mybir.dt.float32
    hs = slice(h0 * C, (h0 + nh) * C)
    dg = npool.tile([P, nh, C], f32, tag=f"dg_{h0}", name=f"dg_{h0}_{q}")
    rc = npool.tile([P, nh, C], f32, tag=f"rc_{h0}", name=f"rc_{h0}_{q}")
    eng = nc.vector if engine == 0 else nc.gpsimd
    dsrc = att[:, q, h0:h0 + nh, C:C1].broadcast_to([P, nh, C])
    eng.tensor_tensor(dg[:], dsrc, g1t[:], op=ALU.mult)
    nc.vector.reciprocal(rc[:], dg[:])
    nsrc = att[:, q, h0:h0 + nh, 0:C]
    dst = ag[q][:, hs].rearrange("p (h c) -> p h c", h=nh)
    eng.tensor_tensor(dst, nsrc, rc[:], op=ALU.mult)


def _norm_pool(nc, mybir, npool, att, g1t, ag, q):
    """h7 normalize with multiplies on Pool (keeps the tail DVE light)."""
    ALU = mybir.AluOpType
    f32 = mybir.dt.float32
    hs = slice(7 * C, 8 * C)
    dg = npool.tile([P, 1, C], f32, tag="dg_t", name=f"dgt{q}")
    rc = npool.tile([P, 1, C], f32, tag="rc_t", name=f"rct{q}")
    dsrc = att[:, q, 7:8, C:C1].broadcast_to([P, 1, C])
    nc.gpsimd.tensor_tensor(dg[:], dsrc, g1t[:], op=ALU.mult)
    nc.vector.reciprocal(rc[:], dg[:])
    nsrc = att[:, q, 7:8, 0:C]
    dst = ag[q][:, hs].rearrange("p (h c) -> p h c", h=1)
    nc.gpsimd.tensor_tensor(dst, nsrc, rc[:], op=ALU.mult)


def _prep_inputs(features, ln_g, ln_b, Wq, bq, Wk, bk, Wv, bv, Wb, bb,
                 Wg, bg, Wo, bo):
    f32 = np.float32
    sq = f32(1.0 / np.sqrt(C))
    g_ = np.asarray(ln_g, f32)[:, None]
    b_ = np.asarray(ln_b, f32)

    def wsplit(W, n):
        return np.ascontiguousarray(
            np.asarray(W, f32).reshape(NFC, P, n).transpose(1, 0, 2))

    def bsplit(b):
        return np.ascontiguousarray(np.asarray(b, f32).reshape(NFC, P).T)

    Wq_ = np.asarray(Wq, f32) * g_ * sq
    bq_ = (b_ @ (np.asarray(Wq, f32) * sq) + np.asarray(bq, f32) * sq)
    Wk_ = np.asarray(Wk, f32) * g_
    bk_ = b_ @ np.asarray(Wk, f32) + np.asarray(bk, f32)
    Wv_ = np.asarray(Wv, f32) * g_
    bv_ = b_ @ np.asarray(Wv, f32) + np.asarray(bv, f32)
    Wg_ = np.asarray(Wg, f32) * g_
    bg_ = b_ @ np.asarray(Wg, f32) + np.asarray(bg, f32)
    Wb_ = np.asarray(Wb, f32) * g_
    bb_ = b_ @ np.asarray(Wb, f32) + np.asarray(bb, f32)

    common = {
        "wq": wsplit(Wq_, HC),
        "wk": wsplit(Wk_, HC),
        "wv": wsplit(Wv_, HC),
        "wg": wsplit(Wg_, HC),
        "wb": wsplit(Wb_, H),
        "wo": wsplit(Wo, F),
        "bq_t": bsplit(bq_),
        "bk_t": bsplit(bk_),
        "bv_r": np.ascontiguousarray(bv_[None, :]),
        "bg_r": np.ascontiguousarray(bg_[None, :]),
        "bo_r": np.ascontiguousarray(np.asarray(bo, f32)[None, :]),
        "bb_b": np.ascontiguousarray(np.tile(bb_, (P, 1))),
        "ident": np.eye(P, dtype=f32),
        "ones1": np.ones((1, P), f32),
    }
    feats = np.asarray(features, f32)
    in_maps = []
    for c_ in range(N_CORES):
        m = dict(common)
        m["feat"] = np.ascontiguousarray(feats[:, c_, :])
        in_maps.append(m)
    return in_maps


def kernel(**inputs):
    from concourse.bass_utils import run_bass_kernel_spmd

    if "nc" not in _COMPILED:
        _COMPILED["nc"] = _build()
    nc = _COMPILED["nc"]
    in_maps = _prep_inputs(**inputs)
    res = run_bass_kernel_spmd(nc, in_maps, list(range(N_CORES)))
    out = np.stack([res.results[c_]["out"] for c_ in range(N_CORES)], axis=1)
    return np.ascontiguousarray(out.astype(np.float32))


if __name__ == "__main__":
    rng = np.random.default_rng(0)
    ins = {
        "features": rng.standard_normal((L, B, F), dtype=np.float32),
        "ln_g": np.ones(F, np.float32), "ln_b": np.zeros(F, np.float32),
        "Wq": rng.standard_normal((F, HC), dtype=np.float32) * 0.02,
        "bq": np.zeros(HC, np.float32),
        "Wk": rng.standard_normal((F, HC), dtype=np.float32) * 0.02,
        "bk": np.zeros(HC, np.float32),
        "Wv": rng.standard_normal((F, HC), dtype=np.float32) * 0.02,
        "bv": np.zeros(HC, np.float32),
        "Wb": rng.standard_normal((F, H), dtype=np.float32) * 0.02,
        "bb": np.zeros(H, np.float32),
        "Wg": rng.standard_normal((F, HC), dtype=np.float32) * 0.02,
        "bg": np.zeros(HC, np.float32),
        "Wo": rng.standard_normal((HC, F), dtype=np.float32) * 0.02,
        "bo": np.zeros(F, np.float32),
    }
    print(kernel(**ins).shape)


# revision 42
# speedup vs baseline: 1.0096x; 1.0094x over previous
"""Trainium2 Bass kernel for nn_PairwiseAttentionTerminal.

Reference computation (L=1024, B=8, F=256, H=8, C=32):
    x = layernorm(features)                       # (L, B, F)
    q,k,v = x@Wq+bq, x@Wk+bk, x@Wv+bv             # (L, B, H, C)
    bias  = x@Wb+bb                               # (L, B, H) per-key bias
    gate  = sigmoid(x@Wg+bg)                      # (L, B, H, C)
    S     = einsum('qbhc,kbhc->qbkh', q, k)/sqrt(C) + bias[None]
    attn  = softmax_k(S) @ v                      # (L, B, H, C)
    out   = (attn*gate) @ Wo + bo                 # (L, B, F)

Sharding: batch B=8 -> one batch element per NeuronCore (8 cores), weights
replicated, no collectives.  Host shards/gathers around one SPMD NEFF.

Per-core engine plan (cost-model driven):
  - ACT is the bottleneck: 64 softmax exps of [128k x 1024q] from PSUM
    (per-key bias = per-partition ACT bias operand).  Everything else is
    arranged to hide under that stream.
  - S^T[k,q] per (head, ktile): 32-contraction f32r matmuls (2 x 512 free).
  - AV restructured as out[q, c]: stationary = eT [128k x 128q] slice (bf16),
    moving = ones-augmented V [128k x 33] (bf16) -> 33-cycle matmuls into a
    single-bank accumulator [128, 8qq, 33]; denominator rides along as col 32.
    AV emission is software-pipelined one (h,kk) step behind the S/exp stream
    so the in-order PE queue never blocks on the current exp.
  - Normalize in q-major layout: DVE reciprocal of D*(1+e^-y) fuses the
    sigmoid gate division; no DRAM broadcast roundtrip.  Heads 0-3 normalize
    under the h4-6 exp stream, 4-6 under h7, only h7 in the tail.
  - gate/v/output biases are rank-1 matmuls (ones[1,128] x bias_row) chained
    into the projection accumulation.
  - ag (gated attn, q-major bf16) -> PE-transposed (bf16 identity, 1 cyc/row)
    -> Wo projection per qtile, pipelined drain+DMA tail.
  - PE heater matmuls at t=0 ramp the PE clock (p-state) before the real
    front (LN -> transpose -> q/k projections) hits it.
"""

import numpy as np
from contextlib import ExitStack

L, B, F, H, C = 1024, 8, 256, 8, 32
HC = H * C
C1 = C + 1
EPS = 1e-5
N_CORES = 8
P = 128
NLT = L // P   # 8 L-tiles (== qtiles == ktiles)
NFC = F // P   # 2 F-chunks
NQT = L // P   # 8 q-tiles

_COMPILED = {}


def _build():
    import concourse.bacc as bacc
    import concourse.mybir as mybir
    import concourse.tile as tile

    f32 = mybir.dt.float32
    f32r = mybir.dt.float32r
    bf16 = mybir.dt.bfloat16
    AF = mybir.ActivationFunctionType
    ALU = mybir.AluOpType

    nc = bacc.Bacc("TRN2", target_bir_lowering=False)

    # ---- DRAM I/O (per-core) ----
    feat_e = nc.dram_tensor("feat", [L, F], f32, kind="ExternalInput")
    wq_e = nc.dram_tensor("wq", [P, NFC, HC], f32r, kind="ExternalInput")
    wk_e = nc.dram_tensor("wk", [P, NFC, HC], f32r, kind="ExternalInput")
    wv_e = nc.dram_tensor("wv", [P, NFC, HC], f32r, kind="ExternalInput")
    wg_e = nc.dram_tensor("wg", [P, NFC, HC], f32r, kind="ExternalInput")
    wb_e = nc.dram_tensor("wb", [P, NFC, H], f32r, kind="ExternalInput")
    wo_e = nc.dram_tensor("wo", [P, NFC, F], f32r, kind="ExternalInput")
    bq_e = nc.dram_tensor("bq_t", [P, NFC], f32, kind="ExternalInput")
    bk_e = nc.dram_tensor("bk_t", [P, NFC], f32, kind="ExternalInput")
    bv_e = nc.dram_tensor("bv_r", [1, HC], f32r, kind="ExternalInput")
    bg_e = nc.dram_tensor("bg_r", [1, HC], f32r, kind="ExternalInput")
    bo_e = nc.dram_tensor("bo_r", [1, F], f32r, kind="ExternalInput")
    bb_e = nc.dram_tensor("bb_b", [P, H], f32, kind="ExternalInput")
    id_e = nc.dram_tensor("ident", [P, P], f32r, kind="ExternalInput")
    ones_e = nc.dram_tensor("ones1", [1, P], f32r, kind="ExternalInput")
    out_e = nc.dram_tensor("out", [L, F], f32, kind="ExternalOutput")

    with tile.TileContext(nc) as tc, ExitStack() as ctx:
        const = ctx.enter_context(tc.tile_pool(name="const", bufs=1))
        main = ctx.enter_context(tc.tile_pool(name="main", bufs=1))
        work = ctx.enter_context(tc.tile_pool(name="work", bufs=4))
        epool = ctx.enter_context(tc.tile_pool(name="epool", bufs=4))
        npool = ctx.enter_context(tc.tile_pool(name="npool", bufs=8))
        opool = ctx.enter_context(tc.tile_pool(name="opool", bufs=4))

        # ---- t=0: heater fuel + ACT table prewarm source ----
        ones512 = const.tile([P, 512], f32, name="ones512")
        nc.vector.memset(ones512[:], 1.0)

        # ---- input DMAs, ordered by first use ----
        ft = [const.tile([P, F], f32, name=f"ft{i}") for i in range(NLT)]
        for i in range(4):
            nc.sync.dma_start(ft[i][:], feat_e.ap()[i * P:(i + 1) * P, :])

        def load(eng, name, ext, shape, dt_=f32):
            t = const.tile(shape, dt_, name=name)
            eng.dma_start(t[:], ext.ap())
            return t

        wq = load(nc.sync, "wq_s", wq_e, [P, NFC, HC], f32r)
        wk = load(nc.sync, "wk_s", wk_e, [P, NFC, HC], f32r)
        bq = load(nc.sync, "bq_s", bq_e, [P, NFC])
        bk = load(nc.sync, "bk_s", bk_e, [P, NFC])
        bvr = load(nc.sync, "bv_s", bv_e, [1, HC], f32r)
        for i in range(4, NLT):
            nc.gpsimd.dma_start(ft[i][:], feat_e.ap()[i * P:(i + 1) * P, :])
        ident = load(nc.gpsimd, "id_s", id_e, [P, P], f32r)
        wb = load(nc.gpsimd, "wb_s", wb_e, [P, NFC, H], f32r)
        ones1 = load(nc.gpsimd, "ones1_s", ones_e, [1, P], f32r)
        wv = load(nc.gpsimd, "wv_s", wv_e, [P, NFC, HC], f32r)
        wg = load(nc.gpsimd, "wg_s", wg_e, [P, NFC, HC], f32r)
        bgr = load(nc.gpsimd, "bg_s", bg_e, [1, HC], f32r)
        wo = load(nc.sync, "wo_s", wo_e, [P, NFC, F], f32r)
        bor = load(nc.sync, "bo_s", bo_e, [1, F], f32r)

        # ACT table prewarm: one Ln on the memset-ones tile loads the
        # combined ln/exp table before the front needs it.
        scr0 = const.tile([P, 2], f32, name="scr0")
        nc.scalar.activation(scr0[:, 0:1], ones512[:, 0:1], AF.Ln)

        epst = const.tile([P, 1], f32, name="epst")
        nc.vector.memset(epst[:], EPS)

        # bf16 identity for the ag transposes (1 cyc/row vs 1.5 for f32r);
        # copied on Pool so it stays off the DVE LN stream
        identf = const.tile([P, P], f32, name="identf")
        nc.gpsimd.tensor_copy(identf[:], ident[:])

        # ---- persistent tiles ----
        xT = [main.tile([P, L], f32r, name=f"xT{j}") for j in range(NFC)]
        qT = [main.tile([P, L], f32r, name=f"qT{j}") for j in range(NFC)]
        kT = [main.tile([P, L], f32r, name=f"kT{j}") for j in range(NFC)]
        vaug = [main.tile([P, H, C1], bf16, name=f"vaug{i}") for i in range(NLT)]
        u = [main.tile([P, H], f32, name=f"u{i}") for i in range(NLT)]
        ge = [main.tile([P, HC], bf16, name=f"ge{q}") for q in range(NQT)]
        g1A = [main.tile([P, 4, C], bf16, name=f"g1A{q}") for q in range(NQT)]
        g1B = [main.tile([P, 3, C], bf16, name=f"g1B{q}") for q in range(NQT)]
        g17 = [main.tile([P, 1, C], bf16, name=f"g17{q}") for q in range(NQT)]
        att = main.tile([P, NQT, H, C1], f32, name="att")
        ag = [main.tile([P, HC], f32, name=f"ag{q}") for q in range(NQT)]
        agT = [main.tile([P, L], f32r, name=f"agT{j}") for j in range(NFC)]

        # ================= Front phase =================
        # psF: ftp 1 bank + fqk 1 + fsm 2 = 4 banks; psW (wave-1) 4 banks.
        psF_cm = tc.tile_pool(name="psF", bufs=1, space="PSUM")
        psF = psF_cm.__enter__()

        def heat(n):
            # heaters ride the fqk slot (same shape as qk psum tiles)
            for _ in range(n):
                hp_ = psF.tile([P, 512], f32, tag="fqk", name="heat", bufs=2)
                nc.tensor.matmul(hp_[:], ones512[:, 0:P].bitcast(f32r),
                                 ones512[:].bitcast(f32r),
                                 start=True, stop=True)

        heat(6)

        # LN per L-tile: stats (DVE-paced) split from finish so the DVE
        # stream never blocks on the cross-engine rstd wait
        def ln_stats(i):
            st = work.tile([P, 8], f32, tag="st", bufs=8)
            nc.vector.bn_stats(st[:, 0:6], ft[i][:])
            nc.vector.bn_aggr(st[:, 6:8], st[:, 0:6])
            # rstd = exp(-0.5*ln(var+eps)) (free: scalar ops cost 0)
            nc.scalar.activation(st[:, 3:4], st[:, 7:8], AF.Ln, bias=epst[:])
            nc.scalar.activation(st[:, 4:5], st[:, 3:4], AF.Exp, scale=-0.5)
            return st

        def ln_fin(i, st):
            xn = work.tile([P, F], f32r, tag="xn")
            nc.vector.tensor_scalar(xn[:], ft[i][:], st[:, 6:7], st[:, 4:5],
                                    op0=ALU.subtract, op1=ALU.mult)
            for j in range(NFC):
                tpw = psF.tile([P, 512], f32r, tag="fqk", name=f"tp{i}_{j}",
                               bufs=2)
                tp = tpw[:, 0:P]
                nc.tensor.transpose(tp, xn[:, j * P:(j + 1) * P], ident[:])
                # alternate drains ACT/DVE (ACT idle during the front)
                if (2 * i + j) % 2 == 0:
                    nc.scalar.activation(xT[j][:, i * P:(i + 1) * P], tp,
                                         AF.Copy)
                else:
                    nc.vector.tensor_copy(xT[j][:, i * P:(i + 1) * P], tp)

        def ln_tile(i):
            ln_fin(i, ln_stats(i))

        # per-key bias projection; u = exp(bias) folds the softmax bias
        # into the ones-augmented V (exps become bias-free -> mergeable)
        def b_tile(i):
            # u = exp(xn@Wb); the +bb term is per-head constant and cancels
            # in the softmax normalization, so it is dropped entirely.
            ls = slice(i * P, (i + 1) * P)
            ps2 = psF.tile([P, HC], f32, tag="fsm", name=f"pb{i}", bufs=2)
            nc.tensor.matmul(ps2[:, 0:H], xT[0][:, ls], wb[:, 0, :],
                             start=True, stop=False)
            nc.tensor.matmul(ps2[:, 0:H], xT[1][:, ls], wb[:, 1, :],
                             start=False, stop=True)
            nc.scalar.activation(u[i][:], ps2[:, 0:H], AF.Exp)

        # v projection: vaug[.,h,c] = (v+bv)*u, vaug[.,h,C] = u
        def v_tile(i):
            ls = slice(i * P, (i + 1) * P)
            ps = psF.tile([P, HC], f32, tag="fsm", name=f"pv{i}", bufs=2)
            nc.tensor.matmul(ps[:], xT[0][:, ls], wv[:, 0, :],
                             start=True, stop=False)
            nc.tensor.matmul(ps[:], xT[1][:, ls], wv[:, 1, :],
                             start=False, stop=False)
            nc.tensor.matmul(ps[:], ones1[0:1, 0:P], bvr[:],
                             start=False, stop=True)
            nc.gpsimd.tensor_copy(
                vaug[i][:, :, C:C1].rearrange("p h one -> p (h one)"), u[i][:])
            ub = u[i][:].rearrange("p (h one) -> p h one", one=1)
            nc.vector.tensor_tensor(
                vaug[i][:, :, 0:C],
                ps[:].rearrange("p (h c) -> p h c", h=H),
                ub.broadcast_to([P, H, C]), op=ALU.mult)

        def g_tile(q):
            ls = slice(q * P, (q + 1) * P)
            ps = psF.tile([P, HC], f32, tag="fsm", name=f"pg{q}", bufs=2)
            nc.tensor.matmul(ps[:], xT[0][:, ls], wg[:, 0, :],
                             start=True, stop=False)
            nc.tensor.matmul(ps[:], xT[1][:, ls], wg[:, 1, :],
                             start=False, stop=False)
            nc.tensor.matmul(ps[:], ones1[0:1, 0:P], bgr[:],
                             start=False, stop=True)
            # ge = exp(-(y+bg)); gate = 1/(1+ge) folded into normalize
            nc.scalar.activation(ge[q][:], ps[:], AF.Exp, scale=-1.0)
            # prefetch g1 = 1 + ge per normalize band (bf16 4x on DVE)
            gv = ge[q][:].rearrange("p (h c) -> p h c", h=H)
            eng = nc.vector if q % 2 == 0 else nc.gpsimd
            eng.tensor_scalar(g1A[q][:], gv[:, 0:4, :], 1.0, None, op0=ALU.add)
            eng.tensor_scalar(g1B[q][:], gv[:, 4:7, :], 1.0, None, op0=ALU.add)
            eng.tensor_scalar(g17[q][:], gv[:, 7:8, :], 1.0, None, op0=ALU.add)

        def qk_m(j, m):
            ms = slice(512 * m, 512 * (m + 1))
            for wi, (w, bvec, dst) in enumerate(((wq, bq, qT), (wk, bk, kT))):
                ps = psF.tile([P, 512], f32, tag="fqk", name=f"p{j}{m}",
                              bufs=2)
                nc.tensor.matmul(ps[:], w[:, 0, j * P:(j + 1) * P],
                                 xT[0][:, ms], start=True, stop=False)
                nc.tensor.matmul(ps[:], w[:, 1, j * P:(j + 1) * P],
                                 xT[1][:, ms], start=False, stop=True)
                if wi == 0 or j == 1:
                    nc.vector.tensor_scalar(dst[j][:, ms], ps[:],
                                            bvec[:, j:j + 1], None, op0=ALU.add)
                else:
                    nc.scalar.activation(dst[j][:, ms], ps[:], AF.Identity,
                                         bias=bvec[:, j:j + 1])

        # ---- front pipeline: LN(0-3) -> qk m0 -> first wave exp ----
        sts = [ln_stats(i) for i in range(4)]
        for i in range(4):
            ln_fin(i, sts[i])
        qk_m(0, 0)

        # ============ Attention: 32 exp-groups of [128 x 2048] ============
        # group (h, kq, mh): S^T halves for ktiles 4kq..4kq+3, q-half mh.
        # One bias-free exp per group; AV partials accumulate back into the
        # group's own (already consumed) psum slot, then drain-add to SBUF.
        psW_cm = tc.tile_pool(name="psW", bufs=1, space="PSUM")
        psW = psW_cm.__enter__()

        WAVE = [(0, 0, 0), (1, 0, 0), (2, 0, 0)]
        groups = list(WAVE)
        groups += [(3, 0, 0)]
        groups += [(h, kq, mh) for h in range(4)
                   for (kq, mh) in ((0, 1), (1, 0), (1, 1))]
        groups += [(h, kq, mh) for h in range(4, 8)
                   for (kq, mh) in ((0, 0), (0, 1), (1, 0), (1, 1))]

        def emit_group(gi, pool, tag):
            h, kq, mh = groups[gi]
            jh, ph = h // 4, 32 * (h % 4)
            hp = slice(ph, ph + 32)
            sp = pool.tile([P, 2048], f32, tag=tag, name=f"sp{gi}")
            for b_ in range(4):
                kk = 4 * kq + b_
                nc.tensor.matmul(sp[:, b_ * 512:(b_ + 1) * 512],
                                 kT[jh][hp, kk * P:(kk + 1) * P],
                                 qT[jh][hp, mh * 512:(mh + 1) * 512],
                                 start=True, stop=True, tile_position=(ph, 0))
            eT = epool.tile([P, 2048], bf16, tag="e", name=f"e{gi}")
            nc.scalar.activation(eT[:], sp[:], AF.Exp)
            return (gi, sp, eT)

        def emit_AV(gi, sp, eT, drain_act=False):
            h, kq, mh = groups[gi]
            av = sp[:, 0:132].rearrange("p (qq c) -> p qq c", qq=4)
            for b_ in range(4):
                kk = 4 * kq + b_
                for ql in range(4):
                    nc.tensor.matmul(
                        av[:, ql, :],
                        eT[:, b_ * 512 + ql * P:b_ * 512 + (ql + 1) * P],
                        vaug[kk][:, h, :],
                        start=(b_ == 0 and ql == 0),
                        stop=(b_ == 3 and ql == 3))
            dst = att[:, mh * 4:(mh + 1) * 4, h, :]
            if kq == 0:
                if drain_act:
                    nc.scalar.activation(dst, av, AF.Copy)
                else:
                    nc.vector.tensor_copy(dst, av)
            else:
                nc.vector.tensor_tensor(dst, dst, av, op=ALU.add)

        # wave-1 (single-slot, serialized): fills the ACT idle window while
        # the rest of the front streams in
        pend = emit_group(0, psW, "w")
        # b/v/u must be emitted before the AV that reads vaug
        for i in range(4):
            b_tile(i)
            v_tile(i)
        emit_AV(*pend)
        ln_tile(4)
        ln_tile(5)
        pend = emit_group(1, psW, "w")
        emit_AV(*pend)
        ln_tile(6)
        ln_tile(7)
        g_tile(0)
        g_tile(1)
        qk_m(0, 1)
        for i in range(4, NLT):
            b_tile(i)
            v_tile(i)
        g_tile(2)
        g_tile(3)
        pend = emit_group(2, psW, "w")
        sp_w2 = pend[1]
        emit_AV(*pend)
        qk_m(1, 0)
        qk_m(1, 1)
        # gates 4-7 ride the consumed wave slot (banks 1-2): their psums
        # are ready right after the wave, so the ge exps run during the
        # pool transition instead of gating it
        for q_ in range(4, NQT):
            ls = slice(q_ * P, (q_ + 1) * P)
            ps = sp_w2[:, 512 + 256 * (q_ - 4):768 + 256 * (q_ - 4)]
            nc.tensor.matmul(ps, xT[0][:, ls], wg[:, 0, :],
                             start=True, stop=False)
            nc.tensor.matmul(ps, xT[1][:, ls], wg[:, 1, :],
                             start=False, stop=False)
            nc.tensor.matmul(ps, ones1[0:1, 0:P], bgr[:],
                             start=False, stop=True)
            nc.scalar.activation(ge[q_][:], ps, AF.Exp, scale=-1.0)
            gv = ge[q_][:].rearrange("p (h c) -> p h c", h=H)
            eng = nc.vector if q_ % 2 == 0 else nc.gpsimd
            eng.tensor_scalar(g1A[q_][:], gv[:, 0:4, :], 1.0, None,
                             op0=ALU.add)
            eng.tensor_scalar(g1B[q_][:], gv[:, 4:7, :], 1.0, None,
                             op0=ALU.add)
            eng.tensor_scalar(g17[q_][:], gv[:, 7:8, :], 1.0, None,
                             op0=ALU.add)
        pend = None

        psW_cm.__exit__(None, None, None)
        psF_cm.__exit__(None, None, None)

        # ---- steady stream ----
        psS_cm = tc.tile_pool(name="psS", bufs=2, space="PSUM")
        psS = psS_cm.__enter__()

        tq0_done = 0
        norm_q = []
        prev_sp = None
        for gi in range(len(WAVE), len(groups)):
            nxt = emit_group(gi, psS, "s")
            if pend is not None:
                pgi = pend[0]
                prev_sp = pend[1]
                emit_AV(*pend)
                # normalize bands, 2 chains per group on Pool so DVE keeps
                # the slot-freeing psum drains moving
                for _ in range(2):
                    if norm_q and norm_q[0][0] <= pgi:
                        _, band, q = norm_q.pop(0)
                        g1t = g1A[q] if band == 0 else g1B[q]
                        _norm(nc, mybir, npool, att, g1t, ag, q,
                              0 if band == 0 else 4, 4 if band == 0 else 3,
                              engine=1)
                if groups[pgi] == (3, 1, 1):
                    norm_q += [(pgi, 0, q) for q in range(NQT)]
                if groups[pgi] == (6, 1, 1):
                    norm_q += [(pgi, 1, q) for q in range(NQT)]
            pend = nxt
            # piggyback jh0 ag-transposes into consumed h4/h5 slots (bank 1
            # is free after that group's exp; bank 0 holds the AV partial)
            h = groups[gi][0]
            if (h in (4, 5, 6) and prev_sp is not None and tq0_done < NQT
                    and tq0_done < 2 * (gi - 16)):
                q = tq0_done
                tq_ap = prev_sp[:, 576:704]
                nc.tensor.transpose(tq_ap, ag[q][:, 0:P], identf[:])
                nc.vector.tensor_copy(agT[0][:, q * P:(q + 1) * P], tq_ap)
                tq0_done += 1
            if gi == len(groups) - 1:
                # flush normalize chains the 2-per-group drain didn't reach
                while norm_q:
                    _, band, q = norm_q.pop(0)
                    g1t = g1A[q] if band == 0 else g1B[q]
                    _norm(nc, mybir, npool, att, g1t, ag, q,
                          0 if band == 0 else 4, 4 if band == 0 else 3,
                          engine=1)
                # ---- tail, fully slot-riding ----
                # qq 0-3 chains use the (7,1,0) slot (prev_sp) while the
                # final exp runs; qq 4-7 use the (7,1,1) slot (pend) after
                # its exp+AV.
                def wo_chain(q, sp_t):
                    ls = slice(q * P, (q + 1) * P)
                    qm = q % 4
                    po = sp_t[:, 1024 + 256 * qm:1280 + 256 * qm]
                    nc.tensor.matmul(po, agT[0][:, ls], wo[:, 0, :],
                                     start=True, stop=False)
                    nc.tensor.matmul(po, agT[1][:, ls], wo[:, 1, :],
                                     start=False, stop=False)
                    nc.tensor.matmul(po, ones1[0:1, 0:P], bor[:],
                                     start=False, stop=True)
                    o = opool.tile([P, F], f32, tag="oo", name=f"oE{q}")
                    if q % 2 == 0:
                        nc.scalar.activation(o[:], po, AF.Copy)
                    else:
                        nc.vector.tensor_copy(o[:], po)
                    (nc.sync if q % 2 == 0 else nc.gpsimd).dma_start(
                        out_e.ap()[ls, :], o[:])

                # qq0-3 chains run during the final exp
                for q in range(4):
                    _norm(nc, mybir, npool, att, g17[q], ag, q, 7, 1,
                          engine=q % 2)
                for q in range(4):
                    ls = slice(q * P, (q + 1) * P)
                    tq_ap = prev_sp[:, 512 + 128 * q:640 + 128 * q]
                    nc.tensor.transpose(tq_ap, ag[q][:, P:2 * P], identf[:])
                    if q % 2 == 0:
                        nc.scalar.activation(agT[1][:, ls], tq_ap, AF.Copy)
                    else:
                        nc.vector.tensor_copy(agT[1][:, ls], tq_ap)
                # final AV + drain the moment the last exp finishes
                sp_last = pend[1]
                emit_AV(*pend)
                pend = None
                for q in range(4, NQT):
                    _norm(nc, mybir, npool, att, g17[q], ag, q, 7, 1,
                          engine=q % 2)
                for q in range(4):
                    wo_chain(q, prev_sp)
                for q in range(4, NQT):
                    ls = slice(q * P, (q + 1) * P)
                    qm = q - 4
                    tq_ap = sp_last[:, 512 + 128 * qm:640 + 128 * qm]
                    nc.tensor.transpose(tq_ap, ag[q][:, P:2 * P], identf[:])
                    if q % 2 == 0:
                        nc.scalar.activation(agT[1][:, ls], tq_ap, AF.Copy)
                    else:
                        nc.vector.tensor_copy(agT[1][:, ls], tq_ap)
                for q in range(4, NQT):
                    wo_chain(q, sp_last)
        psS_cm.__exit__(None, None, None)

    # Restrict Exp/Ln/Square to the combined table so one load serves all.
    import concourse.bacc as bacc_mod
    orig_gat = bacc_mod.get_activation_tables
    AFt = mybir.ActivationFunctionType

    def gat_combined(arch):
        t = orig_gat(arch)
        out = {}
        drop = {AFt.Exp, AFt.Ln, AFt.Square}
        for name, funcs in t.items():
            if name == "natural_log_exp_and_others":
                out[name] = funcs
            else:
                out[name] = funcs - drop
        return out

    bacc_mod.get_activation_tables = gat_combined
    try:
        nc.compile()
    finally:
        bacc_mod.get_activation_tables = orig_gat
    return nc


def _norm(nc, mybir, npool, att, g1t, ag, q, h0, nh, engine):
    """ag[q][:, h0*32:(h0+nh)*32] = N * 1/(D*(1+ge)) for heads h0..h0+nh-1."""
    ALU = mybir.AluOpType
    f32 = mybir.dt.float32
    hs = slice(h0 * C, (h0 + nh) * C)
    dg = npool.tile([P, nh, C], f32, tag=f"dg_{h0}", name=f"dg_{h0}_{q}")
    rc = npool.tile([P, nh, C], f32, tag=f"rc_{h0}", name=f"rc_{h0}_{q}")
    eng = nc.vector if engine == 0 else nc.gpsimd
    dsrc = att[:, q, h0:h0 + nh, C:C1].broadcast_to([P, nh, C])
    eng.tensor_tensor(dg[:], dsrc, g1t[:], op=ALU.mult)
    nc.vector.reciprocal(rc[:], dg[:])
    nsrc = att[:, q, h0:h0 + nh, 0:C]
    dst = ag[q][:, hs].rearrange("p (h c) -> p h c", h=nh)
    eng.tensor_tensor(dst, nsrc, rc[:], op=ALU.mult)


def _norm_pool(nc, mybir, npool, att, g1t, ag, q):
    """h7 normalize with multiplies on Pool (keeps the tail DVE light)."""
    ALU = mybir.AluOpType
    f32 = mybir.dt.float32
    hs = slice(7 * C, 8 * C)
    dg = npool.tile([P, 1, C], f32, tag="dg_t", name=f"dgt{q}")
    rc = npool.tile([P, 1, C], f32, tag="rc_t", name=f"rct{q}")
    dsrc = att[:, q, 7:8, C:C1].broadcast_to([P, 1, C])
    nc.gpsimd.tensor_tensor(dg[:], dsrc, g1t[:], op=ALU.mult)
    nc.vector.reciprocal(rc[:], dg[:])
    nsrc = att[:, q, 7:8, 0:C]
    dst = ag[q][:, hs].rearrange("p (h c) -> p h c", h=1)
    nc.gpsimd.tensor_tensor(dst, nsrc, rc[:], op=ALU.mult)


def _prep_inputs(features, ln_g, ln_b, Wq, bq, Wk, bk, Wv, bv, Wb, bb,
                 Wg, bg, Wo, bo):
    f32 = np.float32
    sq = f32(1.0 / np.sqrt(C))
    g_ = np.asarray(ln_g, f32)[:, None]
    b_ = np.asarray(ln_b, f32)

    def wsplit(W, n):
        return np.ascontiguousarray(
            np.asarray(W, f32).reshape(NFC, P, n).transpose(1, 0, 2))

    def bsplit(b):
        return np.ascontiguousarray(np.asarray(b, f32).reshape(NFC, P).T)

    Wq_ = np.asarray(Wq, f32) * g_ * sq
    bq_ = (b_ @ (np.asarray(Wq, f32) * sq) + np.asarray(bq, f32) * sq)
    Wk_ = np.asarray(Wk, f32) * g_
    bk_ = b_ @ np.asarray(Wk, f32) + np.asarray(bk, f32)
    Wv_ = np.asarray(Wv, f32) * g_
    bv_ = b_ @ np.asarray(Wv, f32) + np.asarray(bv, f32)
    Wg_ = np.asarray(Wg, f32) * g_
    bg_ = b_ @ np.asarray(Wg, f32) + np.asarray(bg, f32)
    Wb_ = np.asarray(Wb, f32) * g_
    bb_ = b_ @ np.asarray(Wb, f32) + np.asarray(bb, f32)

    common = {
        "wq": wsplit(Wq_, HC),
        "wk": wsplit(Wk_, HC),
        "wv": wsplit(Wv_, HC),
        "wg": wsplit(Wg_, HC),
        "wb": wsplit(Wb_, H),
        "wo": wsplit(Wo, F),
        "bq_t": bsplit(bq_),
        "bk_t": bsplit(bk_),
        "bv_r": np.ascontiguousarray(bv_[None, :]),
        "bg_r": np.ascontiguousarray(bg_[None, :]),
        "bo_r": np.ascontiguousarray(np.asarray(bo, f32)[None, :]),
        "bb_b": np.ascontiguousarray(np.tile(bb_, (P, 1))),
        "ident": np.eye(P, dtype=f32),
        "ones1": np.ones((1, P), f32),
    }
    feats = np.asarray(features, f32)
    in_maps = []
    for c_ in range(N_CORES):
        m = dict(common)
        m["feat"] = np.ascontiguousarray(feats[:, c_, :])
        in_maps.append(m)
    return in_maps


def kernel(**inputs):
    from concourse.bass_utils import run_bass_kernel_spmd

    if "nc" not in _COMPILED:
        _COMPILED["nc"] = _build()
    nc = _COMPILED["nc"]
    in_maps = _prep_inputs(**inputs)
    res = run_bass_kernel_spmd(nc, in_maps, list(range(N_CORES)))
    out = np.stack([res.results[c_]["out"] for c_ in range(N_CORES)], axis=1)
    return np.ascontiguousarray(out.astype(np.float32))


if __name__ == "__main__":
    rng = np.random.default_rng(0)
    ins = {
        "features": rng.standard_normal((L, B, F), dtype=np.float32),
        "ln_g": np.ones(F, np.float32), "ln_b": np.zeros(F, np.float32),
        "Wq": rng.standard_normal((F, HC), dtype=np.float32) * 0.02,
        "bq": np.zeros(HC, np.float32),
        "Wk": rng.standard_normal((F, HC), dtype=np.float32) * 0.02,
        "bk": np.zeros(HC, np.float32),
        "Wv": rng.standard_normal((F, HC), dtype=np.float32) * 0.02,
        "bv": np.zeros(HC, np.float32),
        "Wb": rng.standard_normal((F, H), dtype=np.float32) * 0.02,
        "bb": np.zeros(H, np.float32),
        "Wg": rng.standard_normal((F, HC), dtype=np.float32) * 0.02,
        "bg": np.zeros(HC, np.float32),
        "Wo": rng.standard_normal((HC, F), dtype=np.float32) * 0.02,
        "bo": np.zeros(F, np.float32),
    }
    print(kernel(**ins).shape)
